# revision 27
# baseline (speedup 1.0000x reference)
"""BotGCN on 8 Trainium2 NeuronCores (v2: bf16 + bucket-pipelined AllGather).

Strategy (pull-mode GNN message passing):
  - Nodes are relabeled by a degree-balanced permutation and dst-sharded
    across 8 cores (SH=12800 nodes/core = 100 windows of 128).
  - The whole data path runs in bf16 (f32 PSUM accumulation): features,
    weights, h tables, messages, one-hot norms. Final output f32.
  - Phase 1 (per-core): feature MLP in transposed layout x^T [128, nodes]
    via a block-diagonal weight matmul; fused with the h1 = x @ W_g1 table
    build, per src-bucket; each bucket's table shard AllGathers as soon as
    its quarter of phase 1 finishes (4 pipelined collectives per layer).
  - GCN scatter (bucket-outer): per src bucket b, per dst window w, 128-edge
    batches: dma_gather pulls h[src] bf16 rows from table_b; a valued
    one-hot (iota==dst_slot)*norm built on DVE is the matmul rhs; PE
    accumulates out^T[feat, dst] in PSUM; DVE folds PSUM into an SBUF f32
    accumulator [128, SH]. The b==3 fold writes bf16 x^T for the next
    matmul stage, letting table2-build/AG2 overlap the tail of layer 1.
  - Edge idx/meta streams are preloaded per bucket in 3 large DMAs.
  - Src indices are int16 (dma_gather requirement): 4 buckets of 25600
    rows; per (bucket, window) batch counts are padded to the max across
    cores so one program serves all 8 cores.
"""
import os
import sys

sys.path.insert(0, "/opt/trn_rl_repo")

import numpy as np

_KSTAGE = int(os.environ.get("KSTAGE", "5"))  # debug: 1..5 partial builds
_KSUB = os.environ.get("KSUB", "full")  # debug: gcn sublayer variant a|b|full

NCORES = 8
P = 128
NBK = 4          # src buckets (int16 index range)
CH = 4096        # gather chunk size in idxs (32 batches)
NCHUNK = 512     # phase-1/head node chunk (matmul moving free dim)
LRELU_SLOPE = 0.01
SH = 12800       # nodes per core (100 windows)
NP_ = SH * NCORES
W = SH // P      # 100 windows
BSL = SH // NBK  # 3200 per-core rows per bucket
BS = BSL * NCORES  # 25600 global rows per bucket


def _bf16():
    import jax.numpy as jnp
    return jnp.bfloat16


# ---------------------------------------------------------------- host prep

def _preprocess(des, tweet, num_prop, cat_prop, edge_index):
    bf16 = _bf16()
    N = des.shape[0]
    E = edge_index.shape[1]
    assert N <= NP_
    assert BS <= 32767, "src bucket exceeds int16 range"

    src = edge_index[0].astype(np.int64)
    dst = edge_index[1].astype(np.int64)
    deg = (np.bincount(dst, minlength=N) + 1).astype(np.float32)
    dinv = (1.0 / np.sqrt(deg)).astype(np.float32)

    # degree-balanced snake deal of nodes into NCORES*W window bins
    nbins = NCORES * W
    order = np.argsort(-deg, kind="stable")
    i = np.arange(N)
    row, col = i // nbins, i % nbins
    bin_of = np.where(row % 2 == 0, col, nbins - 1 - col)
    new_id = np.empty(N, np.int64)
    new_id[order] = bin_of * P + row
    assert row.max() < P

    ns = np.concatenate([new_id[src], new_id])            # + self loops
    nd = np.concatenate([new_id[dst], new_id])
    nrm = np.concatenate([dinv[src] * dinv[dst], dinv * dinv]).astype(np.float32)

    core = nd // SH
    w = (nd % SH) // P
    slot = nd % P
    cs = ns // SH
    lsb = ns % SH
    bkt = lsb // BSL
    lsrc = cs * BSL + (lsb % BSL)                         # bucket-local idx
    key = (core * NBK + bkt) * W + w
    o = np.argsort(key, kind="stable")
    lsrc_s, slot_s, nrm_s = lsrc[o], slot[o], nrm[o]

    cnt = np.bincount(key[o], minlength=NCORES * NBK * W).reshape(NCORES, NBK, W)
    K = -(-cnt // P)                                      # ceil batches
    Kmax = K.max(axis=0)                                  # [NBK, W]
    glen = Kmax * P
    Lb = glen.sum(axis=1)                                 # [NBK] idxs/bucket
    boff = np.zeros(NBK, np.int64)
    boff[1:] = np.cumsum(Lb)[:-1]
    cum_w = np.zeros((NBK, W), np.int64)
    cum_w[:, 1:] = np.cumsum(glen, axis=1)[:, :-1]
    dstoff = boff[:, None] + cum_w                        # [NBK, W] idx offset
    L = int(Lb.sum())                                     # padded idxs / core
    B_tot = L // P

    starts = np.zeros(NCORES * NBK * W + 1, np.int64)
    starts[1:] = np.cumsum(cnt.reshape(-1))
    lsrc_pad = np.zeros((NCORES, L), np.int16)
    slot_pad = np.zeros((NCORES, L), np.float32)
    nrm_pad = np.zeros((NCORES, L), np.float32)
    for c in range(NCORES):
        for b in range(NBK):
            for wi in range(W):
                f = (c * NBK + b) * W + wi
                n = cnt[c, b, wi]
                if n == 0:
                    continue
                s0, d0 = starts[f], dstoff[b, wi]
                lsrc_pad[c, d0:d0 + n] = lsrc_s[s0:s0 + n]
                slot_pad[c, d0:d0 + n] = slot_s[s0:s0 + n]
                nrm_pad[c, d0:d0 + n] = nrm_s[s0:s0 + n]

    # gather idx layout: idx j -> [j % 16 (replicated x8), j // 16]
    idx_np = np.transpose(lsrc_pad.reshape(NCORES, L // 16, 16), (0, 2, 1))
    idx_np = np.tile(idx_np, (1, 8, 1)).copy()            # [NCORES,128,L/16]
    ldst_np = np.ascontiguousarray(
        np.transpose(slot_pad.reshape(NCORES, B_tot, P), (0, 2, 1))).astype(bf16)
    nrm_np = np.ascontiguousarray(
        np.transpose(nrm_pad.reshape(NCORES, B_tot, P), (0, 2, 1))).astype(bf16)

    # per-core transposed feature blob [KF, SH] bf16
    D1, D2, D3, D4 = des.shape[1], tweet.shape[1], num_prop.shape[1], cat_prop.shape[1]
    KF_raw = D1 + D2 + D3 + D4
    KF = ((KF_raw + P - 1) // P) * P
    inv = np.full(NP_, -1, np.int64)
    inv[new_id] = np.arange(N)
    featT = np.zeros((NCORES, KF, SH), bf16)
    feat_cat = np.concatenate([des, tweet, num_prop, cat_prop], axis=1)
    for c in range(NCORES):
        sel = inv[c * SH:(c + 1) * SH]
        valid = sel >= 0
        block = np.zeros((SH, KF_raw), np.float32)
        block[valid] = feat_cat[sel[valid]]
        featT[c, :KF_raw, :] = block.T.astype(bf16)
    return dict(
        N=N, E=E, KF=KF, L=L, B_tot=B_tot,
        Kmax=Kmax, dstoff=dstoff, boff=boff, Lb=Lb,
        new_id=new_id, featT=featT, idx=idx_np, ldst=ldst_np, nrm=nrm_np,
        D=(D1, D2, D3, D4),
    )


def _weight_blobs(pp, W_des, b_des, W_tweet, b_tweet, W_num, b_num, W_cat, b_cat,
                  W_in, b_in, W_g1, b_g1, W_g2, b_g2, W_o1, b_o1, W_o2, b_o2):
    bf16 = _bf16()
    KF, HID = pp["KF"], W_in.shape[0]
    D1, D2, D3, D4 = pp["D"]
    q = W_des.shape[1]
    wblk = np.zeros((KF, HID), np.float32)
    wblk[0:D1, 0:q] = W_des
    wblk[D1:D1 + D2, q:2 * q] = W_tweet
    wblk[D1 + D2:D1 + D2 + D3, 2 * q:3 * q] = W_num
    wblk[D1 + D2 + D3:D1 + D2 + D3 + D4, 3 * q:4 * q] = W_cat
    wsq = np.concatenate([W_in, W_g1, W_g2, W_o1], axis=1).astype(np.float32)
    wo2 = np.zeros((HID, 2), np.float32)
    wo2[:, :] = W_o2
    b0 = np.concatenate([b_des, b_tweet, b_num, b_cat]).astype(np.float32)
    biases = np.zeros((HID, 6), np.float32)
    biases[:, 0] = b0
    biases[:, 1] = b_in
    biases[:, 2] = b_g1
    biases[:, 3] = b_g2
    biases[:, 4] = b_o1
    biases[:len(b_o2), 5] = b_o2
    bias_nz = [bool(np.any(b != 0)) for b in (b0, b_in, b_g1, b_g2, b_o1, b_o2)]
    iota = np.tile(np.arange(P, dtype=np.float32)[None, :], (P, 1))
    return (wblk.astype(bf16), wsq.astype(bf16), wo2.astype(bf16), biases,
            bias_nz, iota.astype(bf16))


# ---------------------------------------------------------------- device

def _bucket_chunks(b):
    """Node chunks (start, len) covering bucket b, len<=NCHUNK, mult of P."""
    out = []
    c0 = b * BSL
    end = (b + 1) * BSL
    while c0 < end:
        nn = min(NCHUNK, end - c0)
        out.append((c0, nn))
        c0 += nn
    return out


def _build_nc(pp, bias_nz):
    import concourse.bass as bass
    import concourse.bacc as bacc
    import concourse.mybir as mybir
    import concourse.tile as tile

    f32 = mybir.dt.float32
    bf16 = mybir.dt.bfloat16
    i16 = mybir.dt.int16
    KF = pp["KF"]
    L, B_tot = pp["L"], pp["B_tot"]
    Kmax, dstoff, boff = pp["Kmax"], pp["dstoff"], pp["boff"]
    Lb = pp["Lb"]
    HID = 128
    NKC = KF // P                                   # phase-1 K chunks
    Lbmax = int(Lb.max())

    nc = bacc.Bacc("TRN2", target_bir_lowering=False, debug=False,
                   num_devices=NCORES, num_swdge_queues=4)

    featT_d = nc.dram_tensor("featT", [KF, SH], bf16, kind="ExternalInput")
    idx_d = nc.dram_tensor("idx", [P, L // 16], i16, kind="ExternalInput")
    ldst_d = nc.dram_tensor("ldst", [P, B_tot], bf16, kind="ExternalInput")
    nrm_d = nc.dram_tensor("nrm", [P, B_tot], bf16, kind="ExternalInput")
    wblk_d = nc.dram_tensor("wblk", [KF, HID], bf16, kind="ExternalInput")
    wsq_d = nc.dram_tensor("wsq", [HID, 4 * HID], bf16, kind="ExternalInput")
    wo2_d = nc.dram_tensor("wo2", [HID, 2], bf16, kind="ExternalInput")
    bias_d = nc.dram_tensor("biases", [HID, 6], f32, kind="ExternalInput")
    iota_d = nc.dram_tensor("iota", [P, P], bf16, kind="ExternalInput")
    out_d = nc.dram_tensor("out", [2, SH], f32, kind="ExternalOutput")

    with tile.TileContext(nc) as tc:
        with (
            tc.tile_pool(name="const", bufs=1) as constp,
            tc.tile_pool(name="xt", bufs=2) as xtp,
            tc.tile_pool(name="feat", bufs=4) as featp,
            tc.tile_pool(name="tmp", bufs=2) as tmpp,
            tc.tile_pool(name="msgs", bufs=6) as msgp,
            tc.tile_pool(name="idxp", bufs=8) as idxp,
            tc.tile_pool(name="metap", bufs=4) as metap,
            tc.tile_pool(name="eqp", bufs=6) as eqp,
            tc.tile_pool(name="stg", bufs=3) as stgp,
            tc.tile_pool(name="ps", bufs=7, space="PSUM") as psp,
            tc.tile_pool(name="dram", bufs=1, space="DRAM") as dramp,
        ):
            # ---- constants
            wblk = constp.tile([P, NKC, HID], bf16)
            nc.sync.dma_start(wblk[:], wblk_d.ap().rearrange("(a p) c -> p a c", p=P))
            wsq = constp.tile([HID, 4 * HID], bf16)
            nc.sync.dma_start(wsq[:], wsq_d[:])
            w_in, w_g1 = wsq[:, 0:HID], wsq[:, HID:2 * HID]
            w_g2, w_o1 = wsq[:, 2 * HID:3 * HID], wsq[:, 3 * HID:4 * HID]
            wo2 = constp.tile([HID, 2], bf16)
            nc.sync.dma_start(wo2[:], wo2_d[:])
            biases = constp.tile([HID, 6], f32)
            nc.sync.dma_start(biases[:], bias_d[:])
            iota = constp.tile([P, P], bf16)
            nc.sync.dma_start(iota[:], iota_d[:])
            zero128 = constp.tile([P, P], f32)
            nc.vector.memset(zero128[:], 0.0)

            def lrelu(dst_ap, src_ap, bias_idx, nn, tag):
                """dst = lrelu(src + bias); src is PSUM f32, dst bf16."""
                if bias_nz[bias_idx]:
                    u = tmpp.tile([P, NCHUNK], f32, name=f"u{tag}", tag="lrelu_u")
                    nc.vector.tensor_scalar(
                        u[:, :nn], src_ap, biases[:, bias_idx:bias_idx + 1], None,
                        op0=mybir.AluOpType.add)
                    src_ap = u[:, :nn]
                t = tmpp.tile([P, NCHUNK], f32, name=f"t{tag}", tag="lrelu_t")
                nc.vector.tensor_scalar(
                    t[:, :nn], src_ap, LRELU_SLOPE, None, op0=mybir.AluOpType.mult)
                nc.vector.tensor_tensor(
                    dst_ap, src_ap, t[:, :nn], op=mybir.AluOpType.max)

            # ---- phase 1 fused with table-1 build, bucket by bucket
            x1T = xtp.tile([P, SH], bf16, name="x1T", tag="xT")
            hloc1 = [dramp.tile([BSL, HID], bf16, name=f"hloc1_{b}")
                     for b in range(NBK)]
            table1 = [dramp.tile([BS, HID], bf16, addr_space="Shared",
                                 name=f"table1_{b}") for b in range(NBK)]
            for b in range(NBK):
                for (c0, nn) in _bucket_chunks(b):
                    px = psp.tile([P, NCHUNK], f32, name="px", tag="ps")
                    for k in range(NKC):
                        ft = featp.tile([P, NCHUNK], bf16, name="ft", tag="feat")
                        nc.sync.dma_start(ft[:, :nn],
                                          featT_d[k * P:(k + 1) * P, c0:c0 + nn])
                        nc.tensor.matmul(px[:, :nn], wblk[:, k, :], ft[:, :nn],
                                         start=(k == 0), stop=(k == NKC - 1))
                    x0 = tmpp.tile([P, NCHUNK], bf16, name="x0", tag="x0")
                    lrelu(x0[:, :nn], px[:, :nn], 0, nn, "p1")
                    px1 = psp.tile([P, NCHUNK], f32, name="px1", tag="ps")
                    nc.tensor.matmul(px1[:, :nn], w_in, x0[:, :nn],
                                     start=True, stop=True)
                    lrelu(x1T[:, c0:c0 + nn], px1[:, :nn], 1, nn, "p2")
                    if _KSTAGE >= 2:
                        ph = psp.tile([P, NCHUNK], f32, name="ph1", tag="ps")
                        for s in range(nn // P):
                            nc.tensor.matmul(ph[:, s * P:(s + 1) * P],
                                             x1T[:, c0 + s * P:c0 + (s + 1) * P],
                                             w_g1, start=True, stop=True)
                        g = nn // P
                        hs = stgp.tile([P, NCHUNK // P, P], bf16, name="hs1",
                                       tag="hs")
                        nc.vector.tensor_copy(
                            hs[:, :g, :].rearrange("p a b -> p (a b)"), ph[:, :nn])
                        l0 = c0 - b * BSL
                        nc.sync.dma_start(
                            hloc1[b][l0:l0 + nn, :].rearrange(
                                "(g p) f -> p g f", p=P),
                            hs[:, :g, :])
                if _KSTAGE >= 2:
                    nc.gpsimd.collective_compute(
                        "AllGather", mybir.AluOpType.bypass,
                        ins=[hloc1[b].opt()], outs=[table1[b].opt()],
                        replica_groups=[list(range(NCORES))],
                    )

            # ---- gcn scatter layer (window-outer, PSUM-only accumulation)
            # 4 windows share one PSUM bank [128, 512]; each window's batches
            # (across all 4 src buckets) chain-accumulate into its quarter;
            # one DVE cast-out per bank. All DVE ops read only DMA-fed or
            # const tiles -- no DVE->DVE RAW chains (those cost ~4us each).
            def gcn_layer(tables, xT_out, bias_idx, lname):
                qn = [0]
                lts, nts = [], []
                for b in range(NBK):
                    nbb = int(Lb[b]) // P                  # batches in bucket
                    bb0 = int(boff[b]) // P
                    lt = metap.tile([P, Lbmax // P], bf16, name=f"lt{lname}",
                                    tag="ldst")
                    nc.sync.dma_start(lt[:, :nbb], ldst_d[:, bb0:bb0 + nbb])
                    nt = metap.tile([P, Lbmax // P], bf16, name=f"nt{lname}",
                                    tag="nrm")
                    nc.sync.dma_start(nt[:, :nbb], nrm_d[:, bb0:bb0 + nbb])
                    lts.append(lt)
                    nts.append(nt)

                caches = [{} for _ in range(NBK)]

                def get_chunk(b, ci):
                    if ci in caches[b]:
                        return caches[b][ci]
                    start = ci * CH                        # bucket-local
                    size = min(CH, int(Lb[b]) - start)
                    nb = size // P
                    mt = None
                    if _KSUB != "b":
                        it = idxp.tile([P, CH // 16], i16, name=f"it{lname}",
                                       tag="idx")
                        nc.sync.dma_start(
                            it[:, :size // 16],
                            idx_d[:, (int(boff[b]) + start) // 16:
                                  (int(boff[b]) + start + size) // 16])
                        mt = msgp.tile([P, CH // P, P], bf16,
                                       name=f"mt{lname}", tag="msgs")
                        nc.gpsimd.dma_gather(
                            mt[:, :nb, :], tables[b][:],
                            it[:, :size // 16],
                            num_idxs=size, num_idxs_reg=size, elem_size=HID,
                            single_packet=False, queue_num=qn[0] % 4)
                        qn[0] += 1
                        # fold norm into messages (in place; no DVE-DVE RAW)
                        c0 = ci * (CH // P)
                        nt_b = nts[b][:, c0:c0 + nb].rearrange(
                            "p (b x) -> p b x", x=1).broadcast_to([P, nb, P])
                        nc.vector.tensor_tensor(mt[:, :nb, :], mt[:, :nb, :],
                                                nt_b, op=mybir.AluOpType.mult)
                    # pure one-hot: eq[p, i, j] = (j == lt[p, c0+i])
                    c0 = ci * (CH // P)
                    iota_b = iota[:].rearrange(
                        "p (b x) -> p b x", b=1).broadcast_to([P, nb, P])
                    lt_b = lts[b][:, c0:c0 + nb].rearrange(
                        "p (b x) -> p b x", x=1).broadcast_to([P, nb, P])
                    eq = eqp.tile([P, CH // P, P], bf16, name=f"eq{lname}",
                                  tag="eq")
                    nc.vector.tensor_tensor(eq[:, :nb, :], iota_b, lt_b,
                                            op=mybir.AluOpType.is_equal)
                    caches[b][ci] = (mt, eq)
                    return caches[b][ci]

                NG = 4                                     # windows per bank
                for g0 in range(0, W, NG):
                    gw = min(NG, W - g0)
                    pw = psp.tile([P, NG * P], f32, name=f"pw{lname}", tag="ps")
                    for j in range(gw):
                        wi = g0 + j
                        nbatch = int(Kmax[:, wi].sum())
                        assert nbatch > 0
                        done = 0
                        for b in range(NBK):
                            posb0 = int(dstoff[b, wi]) - int(boff[b])
                            for k in range(int(Kmax[b, wi])):
                                pos = posb0 + k * P
                                mt, eq = get_chunk(b, pos // CH)
                                i_in = (pos % CH) // P
                                lhs = (iota[:] if _KSUB in ("b", "c")
                                       else mt[:, i_in, :])
                                rhs = (eq[:, i_in, :] if _KSUB != "c"
                                       else iota[:])
                                nc.tensor.matmul(
                                    pw[:, j * P:(j + 1) * P], lhs, rhs,
                                    start=(done == 0),
                                    stop=(done == nbatch - 1))
                                done += 1
                    osl = xT_out[:, g0 * P:(g0 + gw) * P]
                    if bias_nz[bias_idx]:
                        nc.vector.tensor_scalar(
                            osl, pw[:, :gw * P],
                            biases[:, bias_idx:bias_idx + 1], None,
                            op0=mybir.AluOpType.add)
                    else:
                        nc.vector.tensor_copy(osl, pw[:, :gw * P])

            # ---- table build for layer 2 (+ per-bucket AllGather)
            def build_table2(xT, w_g, lname):
                hloc = [dramp.tile([BSL, HID], bf16, name=f"hloc{lname}_{b}")
                        for b in range(NBK)]
                table = [dramp.tile([BS, HID], bf16, addr_space="Shared",
                                    name=f"table{lname}_{b}")
                         for b in range(NBK)]
                if _KSUB == "j":
                    return table
                for b in range(NBK):
                    for (c0, nn) in _bucket_chunks(b):
                        ph = psp.tile([P, NCHUNK], f32, name=f"ph{lname}",
                                      tag="ps")
                        for s in range(nn // P):
                            nc.tensor.matmul(ph[:, s * P:(s + 1) * P],
                                             xT[:, c0 + s * P:c0 + (s + 1) * P],
                                             w_g, start=True, stop=True)
                        g = nn // P
                        hs = stgp.tile([P, NCHUNK // P, P], bf16,
                                       name=f"hs{lname}", tag="hs")
                        nc.vector.tensor_copy(
                            hs[:, :g, :].rearrange("p a b -> p (a b)"),
                            ph[:, :nn])
                        l0 = c0 - b * BSL
                        nc.sync.dma_start(
                            hloc[b][l0:l0 + nn, :].rearrange(
                                "(g p) f -> p g f", p=P),
                            hs[:, :g, :])
                    if _KSUB not in ("i", "j"):
                        nc.gpsimd.collective_compute(
                            "AllGather", mybir.AluOpType.bypass,
                            ins=[hloc[b].opt()], outs=[table[b].opt()],
                            replica_groups=[list(range(NCORES))],
                        )
                return table

            x3T = x1T
            if _KSTAGE >= 3:
                x2T = xtp.tile([P, SH], bf16, name="x2T", tag="xT")
                gcn_layer(table1, x2T, 2, "1")
                x3T = x2T
            if _KSTAGE >= 4:
                table2 = build_table2(x2T, w_g2, "2")
            if _KSTAGE >= 5:
                x3T = xtp.tile([P, SH], bf16, name="x3T", tag="xT")
                gcn_layer(table2, x3T, 3, "2")

            # ---- head: out = W_o2.T @ lrelu(W_o1.T @ x3T + b_o1) + b_o2
            for b in range(NBK):
                for (c0, nn) in _bucket_chunks(b):
                    py = psp.tile([P, NCHUNK], f32, name="py", tag="ps")
                    nc.tensor.matmul(py[:, :nn], w_o1, x3T[:, c0:c0 + nn],
                                     start=True, stop=True)
                    y = tmpp.tile([P, NCHUNK], bf16, name="y", tag="x0")
                    lrelu(y[:, :nn], py[:, :nn], 4, nn, "hd")
                    po = psp.tile([2, NCHUNK], f32, name="po", tag="ps")
                    nc.tensor.matmul(po[:, :nn], wo2[:], y[:, :nn],
                                     start=True, stop=True)
                    ostg = stgp.tile([2, NCHUNK], f32, name="ostg", tag="ostg")
                    if bias_nz[5]:
                        nc.vector.tensor_scalar(
                            ostg[:, :nn], po[:, :nn], biases[:2, 5:6], None,
                            op0=mybir.AluOpType.add)
                    else:
                        nc.vector.tensor_copy(ostg[:, :nn], po[:, :nn])
                    nc.sync.dma_start(out_d[:, c0:c0 + nn], ostg[:, :nn])

    nc.compile()
    return nc


# ---------------------------------------------------------------- entry

_CACHE = {}
_LAST_PP = None
_LAST_INMAPS = None


def _run(inputs, trace=False):
    global _LAST_PP, _LAST_INMAPS
    from concourse.bass_utils import run_bass_kernel_spmd

    pp = _preprocess(inputs["des"], inputs["tweet"], inputs["num_prop"],
                     inputs["cat_prop"], inputs["edge_index"])
    wblk, wsq, wo2, biases, bias_nz, iota = _weight_blobs(
        pp, inputs["W_des"], inputs["b_des"], inputs["W_tweet"], inputs["b_tweet"],
        inputs["W_num"], inputs["b_num"], inputs["W_cat"], inputs["b_cat"],
        inputs["W_in"], inputs["b_in"], inputs["W_g1"], inputs["b_g1"],
        inputs["W_g2"], inputs["b_g2"], inputs["W_o1"], inputs["b_o1"],
        inputs["W_o2"], inputs["b_o2"])

    key = (pp["N"], pp["E"], tuple(bias_nz), pp["L"], pp["B_tot"])
    if key not in _CACHE:
        _CACHE[key] = _build_nc(pp, bias_nz)
    nc = _CACHE[key]

    in_maps = []
    for c in range(NCORES):
        in_maps.append({
            "featT": pp["featT"][c], "idx": pp["idx"][c],
            "ldst": pp["ldst"][c], "nrm": pp["nrm"][c],
            "wblk": wblk, "wsq": wsq, "wo2": wo2, "biases": biases,
            "iota": iota,
        })
    _LAST_PP, _LAST_INMAPS = pp, in_maps
    res = run_bass_kernel_spmd(nc, in_maps, core_ids=list(range(NCORES)),
                               trace=trace)

    N = pp["N"]
    out_new = np.concatenate(
        [res.results[c]["out"].T for c in range(NCORES)], axis=0)  # [NP, 2]
    out = out_new[pp["new_id"], :]                                  # [N, 2]
    return np.ascontiguousarray(out.astype(np.float32)), res


def kernel(**inputs) -> np.ndarray:
    out, _ = _run(inputs, trace=False)
    return out


# revision 29
# speedup vs baseline: 1.4140x; 1.4140x over previous
"""BotGCN on 8 Trainium2 NeuronCores (v2: bf16 + bucket-pipelined AllGather).

Strategy (pull-mode GNN message passing):
  - Nodes are relabeled by a degree-balanced permutation and dst-sharded
    across 8 cores (SH=12800 nodes/core = 100 windows of 128).
  - The whole data path runs in bf16 (f32 PSUM accumulation): features,
    weights, h tables, messages, one-hot norms. Final output f32.
  - Phase 1 (per-core): feature MLP in transposed layout x^T [128, nodes]
    via a block-diagonal weight matmul; fused with the h1 = x @ W_g1 table
    build, per src-bucket; each bucket's table shard AllGathers as soon as
    its quarter of phase 1 finishes (4 pipelined collectives per layer).
  - GCN scatter (bucket-outer): per src bucket b, per dst window w, 128-edge
    batches: dma_gather pulls h[src] bf16 rows from table_b; a valued
    one-hot (iota==dst_slot)*norm built on DVE is the matmul rhs; PE
    accumulates out^T[feat, dst] in PSUM; DVE folds PSUM into an SBUF f32
    accumulator [128, SH]. The b==3 fold writes bf16 x^T for the next
    matmul stage, letting table2-build/AG2 overlap the tail of layer 1.
  - Edge idx/meta streams are preloaded per bucket in 3 large DMAs.
  - Src indices are int16 (dma_gather requirement): 4 buckets of 25600
    rows; per (bucket, window) batch counts are padded to the max across
    cores so one program serves all 8 cores.
"""
import os
import sys

sys.path.insert(0, "/opt/trn_rl_repo")

import numpy as np

_KSTAGE = int(os.environ.get("KSTAGE", "5"))  # debug: 1..5 partial builds
_KSUB = os.environ.get("KSUB", "full")  # debug: gcn sublayer variant a|b|full

NCORES = 8
P = 128
NBK = 4          # src buckets (int16 index range)
CH = 2048        # gather chunk size in idxs (16 batches)
NCHUNK = 512     # phase-1/head node chunk (matmul moving free dim)
LRELU_SLOPE = 0.01
SH = 12800       # nodes per core (100 windows)
NP_ = SH * NCORES
W = SH // P      # 100 windows
BSL = SH // NBK  # 3200 per-core rows per bucket
BS = BSL * NCORES  # 25600 global rows per bucket


def _bf16():
    import jax.numpy as jnp
    return jnp.bfloat16


# ---------------------------------------------------------------- host prep

def _preprocess(des, tweet, num_prop, cat_prop, edge_index):
    bf16 = _bf16()
    N = des.shape[0]
    E = edge_index.shape[1]
    assert N <= NP_
    assert BS <= 32767, "src bucket exceeds int16 range"

    src = edge_index[0].astype(np.int64)
    dst = edge_index[1].astype(np.int64)
    deg = (np.bincount(dst, minlength=N) + 1).astype(np.float32)
    dinv = (1.0 / np.sqrt(deg)).astype(np.float32)

    # degree-balanced snake deal of nodes into NCORES*W window bins
    nbins = NCORES * W
    order = np.argsort(-deg, kind="stable")
    i = np.arange(N)
    row, col = i // nbins, i % nbins
    bin_of = np.where(row % 2 == 0, col, nbins - 1 - col)
    new_id = np.empty(N, np.int64)
    new_id[order] = bin_of * P + row
    assert row.max() < P

    ns = np.concatenate([new_id[src], new_id])            # + self loops
    nd = np.concatenate([new_id[dst], new_id])
    nrm = np.concatenate([dinv[src] * dinv[dst], dinv * dinv]).astype(np.float32)

    core = nd // SH
    w = (nd % SH) // P
    slot = nd % P
    cs = ns // SH
    lsb = ns % SH
    bkt = lsb // BSL
    lsrc = cs * BSL + (lsb % BSL)                         # bucket-local idx
    key = (core * NBK + bkt) * W + w
    o = np.argsort(key, kind="stable")
    lsrc_s, slot_s, nrm_s = lsrc[o], slot[o], nrm[o]

    cnt = np.bincount(key[o], minlength=NCORES * NBK * W).reshape(NCORES, NBK, W)
    K = -(-cnt // P)                                      # ceil batches
    Kmax = K.max(axis=0)                                  # [NBK, W]
    glen = Kmax * P
    Lb = glen.sum(axis=1)                                 # [NBK] idxs/bucket
    boff = np.zeros(NBK, np.int64)
    boff[1:] = np.cumsum(Lb)[:-1]
    cum_w = np.zeros((NBK, W), np.int64)
    cum_w[:, 1:] = np.cumsum(glen, axis=1)[:, :-1]
    dstoff = boff[:, None] + cum_w                        # [NBK, W] idx offset
    L = int(Lb.sum())                                     # padded idxs / core
    B_tot = L // P

    starts = np.zeros(NCORES * NBK * W + 1, np.int64)
    starts[1:] = np.cumsum(cnt.reshape(-1))
    lsrc_pad = np.zeros((NCORES, L), np.int16)
    slot_pad = np.zeros((NCORES, L), np.float32)
    nrm_pad = np.zeros((NCORES, L), np.float32)
    for c in range(NCORES):
        for b in range(NBK):
            for wi in range(W):
                f = (c * NBK + b) * W + wi
                n = cnt[c, b, wi]
                if n == 0:
                    continue
                s0, d0 = starts[f], dstoff[b, wi]
                lsrc_pad[c, d0:d0 + n] = lsrc_s[s0:s0 + n]
                slot_pad[c, d0:d0 + n] = slot_s[s0:s0 + n]
                nrm_pad[c, d0:d0 + n] = nrm_s[s0:s0 + n]

    # gather idx layout: idx j -> [j % 16 (replicated x8), j // 16]
    idx_np = np.transpose(lsrc_pad.reshape(NCORES, L // 16, 16), (0, 2, 1))
    idx_np = np.tile(idx_np, (1, 8, 1)).copy()            # [NCORES,128,L/16]
    ldst_np = np.ascontiguousarray(
        np.transpose(slot_pad.reshape(NCORES, B_tot, P), (0, 2, 1))).astype(bf16)
    nrm_np = np.ascontiguousarray(
        np.transpose(nrm_pad.reshape(NCORES, B_tot, P), (0, 2, 1))).astype(bf16)

    # per-core transposed feature blob [KF, SH] bf16
    D1, D2, D3, D4 = des.shape[1], tweet.shape[1], num_prop.shape[1], cat_prop.shape[1]
    KF_raw = D1 + D2 + D3 + D4
    KF = ((KF_raw + P - 1) // P) * P
    inv = np.full(NP_, -1, np.int64)
    inv[new_id] = np.arange(N)
    featT = np.zeros((NCORES, KF, SH), bf16)
    feat_cat = np.concatenate([des, tweet, num_prop, cat_prop], axis=1)
    for c in range(NCORES):
        sel = inv[c * SH:(c + 1) * SH]
        valid = sel >= 0
        block = np.zeros((SH, KF_raw), np.float32)
        block[valid] = feat_cat[sel[valid]]
        featT[c, :KF_raw, :] = block.T.astype(bf16)
    return dict(
        N=N, E=E, KF=KF, L=L, B_tot=B_tot,
        Kmax=Kmax, dstoff=dstoff, boff=boff, Lb=Lb,
        new_id=new_id, featT=featT, idx=idx_np, ldst=ldst_np, nrm=nrm_np,
        D=(D1, D2, D3, D4),
    )


def _weight_blobs(pp, W_des, b_des, W_tweet, b_tweet, W_num, b_num, W_cat, b_cat,
                  W_in, b_in, W_g1, b_g1, W_g2, b_g2, W_o1, b_o1, W_o2, b_o2):
    bf16 = _bf16()
    KF, HID = pp["KF"], W_in.shape[0]
    D1, D2, D3, D4 = pp["D"]
    q = W_des.shape[1]
    wblk = np.zeros((KF, HID), np.float32)
    wblk[0:D1, 0:q] = W_des
    wblk[D1:D1 + D2, q:2 * q] = W_tweet
    wblk[D1 + D2:D1 + D2 + D3, 2 * q:3 * q] = W_num
    wblk[D1 + D2 + D3:D1 + D2 + D3 + D4, 3 * q:4 * q] = W_cat
    wsq = np.concatenate([W_in, W_g1, W_g2, W_o1], axis=1).astype(np.float32)
    wo2 = np.zeros((HID, 2), np.float32)
    wo2[:, :] = W_o2
    b0 = np.concatenate([b_des, b_tweet, b_num, b_cat]).astype(np.float32)
    biases = np.zeros((HID, 6), np.float32)
    biases[:, 0] = b0
    biases[:, 1] = b_in
    biases[:, 2] = b_g1
    biases[:, 3] = b_g2
    biases[:, 4] = b_o1
    biases[:len(b_o2), 5] = b_o2
    bias_nz = [bool(np.any(b != 0)) for b in (b0, b_in, b_g1, b_g2, b_o1, b_o2)]
    iota = np.tile(np.arange(P, dtype=np.float32)[None, :], (P, 1))
    return (wblk.astype(bf16), wsq.astype(bf16), wo2.astype(bf16), biases,
            bias_nz, iota.astype(bf16))


# ---------------------------------------------------------------- device

def _bucket_chunks(b):
    """Node chunks (start, len) covering bucket b, len<=NCHUNK, mult of P."""
    out = []
    c0 = b * BSL
    end = (b + 1) * BSL
    while c0 < end:
        nn = min(NCHUNK, end - c0)
        out.append((c0, nn))
        c0 += nn
    return out


def _build_nc(pp, bias_nz):
    import concourse.bass as bass
    import concourse.bacc as bacc
    import concourse.mybir as mybir
    import concourse.tile as tile

    f32 = mybir.dt.float32
    bf16 = mybir.dt.bfloat16
    i16 = mybir.dt.int16
    KF = pp["KF"]
    L, B_tot = pp["L"], pp["B_tot"]
    Kmax, dstoff, boff = pp["Kmax"], pp["dstoff"], pp["boff"]
    Lb = pp["Lb"]
    HID = 128
    NKC = KF // P                                   # phase-1 K chunks
    Lbmax = int(Lb.max())

    nc = bacc.Bacc("TRN2", target_bir_lowering=False, debug=False,
                   num_devices=NCORES, num_swdge_queues=4)

    featT_d = nc.dram_tensor("featT", [KF, SH], bf16, kind="ExternalInput")
    idx_d = nc.dram_tensor("idx", [P, L // 16], i16, kind="ExternalInput")
    ldst_d = nc.dram_tensor("ldst", [P, B_tot], bf16, kind="ExternalInput")
    nrm_d = nc.dram_tensor("nrm", [P, B_tot], bf16, kind="ExternalInput")
    wblk_d = nc.dram_tensor("wblk", [KF, HID], bf16, kind="ExternalInput")
    wsq_d = nc.dram_tensor("wsq", [HID, 4 * HID], bf16, kind="ExternalInput")
    wo2_d = nc.dram_tensor("wo2", [HID, 2], bf16, kind="ExternalInput")
    bias_d = nc.dram_tensor("biases", [HID, 6], f32, kind="ExternalInput")
    iota_d = nc.dram_tensor("iota", [P, P], bf16, kind="ExternalInput")
    out_d = nc.dram_tensor("out", [2, SH], f32, kind="ExternalOutput")

    with tile.TileContext(nc) as tc:
        with (
            tc.tile_pool(name="const", bufs=1) as constp,
            tc.tile_pool(name="xt", bufs=2) as xtp,
            tc.tile_pool(name="feat", bufs=4) as featp,
            tc.tile_pool(name="tmp", bufs=2) as tmpp,
            tc.tile_pool(name="msgs", bufs=10) as msgp,
            tc.tile_pool(name="idxp", bufs=10) as idxp,
            tc.tile_pool(name="metap", bufs=4) as metap,
            tc.tile_pool(name="eqp", bufs=10) as eqp,
            tc.tile_pool(name="stg", bufs=3) as stgp,
            tc.tile_pool(name="ps", bufs=7, space="PSUM") as psp,
            tc.tile_pool(name="dram", bufs=1, space="DRAM") as dramp,
        ):
            # ---- constants
            wblk = constp.tile([P, NKC, HID], bf16)
            nc.sync.dma_start(wblk[:], wblk_d.ap().rearrange("(a p) c -> p a c", p=P))
            wsq = constp.tile([HID, 4 * HID], bf16)
            nc.sync.dma_start(wsq[:], wsq_d[:])
            w_in, w_g1 = wsq[:, 0:HID], wsq[:, HID:2 * HID]
            w_g2, w_o1 = wsq[:, 2 * HID:3 * HID], wsq[:, 3 * HID:4 * HID]
            wo2 = constp.tile([HID, 2], bf16)
            nc.sync.dma_start(wo2[:], wo2_d[:])
            biases = constp.tile([HID, 6], f32)
            nc.sync.dma_start(biases[:], bias_d[:])
            iota = constp.tile([P, P], bf16)
            nc.sync.dma_start(iota[:], iota_d[:])
            zero128 = constp.tile([P, P], f32)
            nc.vector.memset(zero128[:], 0.0)

            def lrelu(dst_ap, src_ap, bias_idx, nn, tag):
                """dst = lrelu(src + bias); src is PSUM f32, dst bf16."""
                if bias_nz[bias_idx]:
                    u = tmpp.tile([P, NCHUNK], f32, name=f"u{tag}", tag="lrelu_u")
                    nc.vector.tensor_scalar(
                        u[:, :nn], src_ap, biases[:, bias_idx:bias_idx + 1], None,
                        op0=mybir.AluOpType.add)
                    src_ap = u[:, :nn]
                t = tmpp.tile([P, NCHUNK], f32, name=f"t{tag}", tag="lrelu_t")
                nc.vector.tensor_scalar(
                    t[:, :nn], src_ap, LRELU_SLOPE, None, op0=mybir.AluOpType.mult)
                nc.vector.tensor_tensor(
                    dst_ap, src_ap, t[:, :nn], op=mybir.AluOpType.max)

            # ---- phase 1 fused with table-1 build, bucket by bucket
            x1T = xtp.tile([P, SH], bf16, name="x1T", tag="xT")
            hloc1 = [dramp.tile([BSL, HID], bf16, name=f"hloc1_{b}")
                     for b in range(NBK)]
            table1 = [dramp.tile([BS, HID], bf16, addr_space="Shared",
                                 name=f"table1_{b}") for b in range(NBK)]
            for b in range(NBK):
                for (c0, nn) in _bucket_chunks(b):
                    px = psp.tile([P, NCHUNK], f32, name="px", tag="ps")
                    for k in range(NKC):
                        ft = featp.tile([P, NCHUNK], bf16, name="ft", tag="feat")
                        nc.sync.dma_start(ft[:, :nn],
                                          featT_d[k * P:(k + 1) * P, c0:c0 + nn])
                        nc.tensor.matmul(px[:, :nn], wblk[:, k, :], ft[:, :nn],
                                         start=(k == 0), stop=(k == NKC - 1))
                    x0 = tmpp.tile([P, NCHUNK], bf16, name="x0", tag="x0")
                    lrelu(x0[:, :nn], px[:, :nn], 0, nn, "p1")
                    px1 = psp.tile([P, NCHUNK], f32, name="px1", tag="ps")
                    nc.tensor.matmul(px1[:, :nn], w_in, x0[:, :nn],
                                     start=True, stop=True)
                    lrelu(x1T[:, c0:c0 + nn], px1[:, :nn], 1, nn, "p2")
                    if _KSTAGE >= 2:
                        ph = psp.tile([P, NCHUNK], f32, name="ph1", tag="ps")
                        for s in range(nn // P):
                            nc.tensor.matmul(ph[:, s * P:(s + 1) * P],
                                             x1T[:, c0 + s * P:c0 + (s + 1) * P],
                                             w_g1, start=True, stop=True)
                        g = nn // P
                        hs = stgp.tile([P, NCHUNK // P, P], bf16, name="hs1",
                                       tag="hs")
                        nc.vector.tensor_copy(
                            hs[:, :g, :].rearrange("p a b -> p (a b)"), ph[:, :nn])
                        l0 = c0 - b * BSL
                        nc.sync.dma_start(
                            hloc1[b][l0:l0 + nn, :].rearrange(
                                "(g p) f -> p g f", p=P),
                            hs[:, :g, :])
                if _KSTAGE >= 2:
                    nc.gpsimd.collective_compute(
                        "AllGather", mybir.AluOpType.bypass,
                        ins=[hloc1[b].opt()], outs=[table1[b].opt()],
                        replica_groups=[list(range(NCORES))],
                    )

            # ---- gcn scatter layer (window-outer, PSUM-only accumulation)
            # 4 windows share one PSUM bank [128, 512]; each window's batches
            # (across all 4 src buckets) chain-accumulate into its quarter;
            # one DVE cast-out per bank. All DVE ops read only DMA-fed or
            # const tiles -- no DVE->DVE RAW chains (those cost ~4us each).
            def gcn_layer(tables, xT_out, bias_idx, lname):
                qn = [0]
                lts, nts = [], []
                for b in range(NBK):
                    nbb = int(Lb[b]) // P                  # batches in bucket
                    bb0 = int(boff[b]) // P
                    lt = metap.tile([P, Lbmax // P], bf16, name=f"lt{lname}",
                                    tag="ldst")
                    nc.sync.dma_start(lt[:, :nbb], ldst_d[:, bb0:bb0 + nbb])
                    nt = metap.tile([P, Lbmax // P], bf16, name=f"nt{lname}",
                                    tag="nrm")
                    nc.sync.dma_start(nt[:, :nbb], nrm_d[:, bb0:bb0 + nbb])
                    lts.append(lt)
                    nts.append(nt)

                caches = [{} for _ in range(NBK)]

                def get_chunk(b, ci):
                    if ci in caches[b]:
                        return caches[b][ci]
                    start = ci * CH                        # bucket-local
                    size = min(CH, int(Lb[b]) - start)
                    nb = size // P
                    mt = None
                    if _KSUB != "b":
                        it = idxp.tile([P, CH // 16], i16, name=f"it{lname}",
                                       tag="idx")
                        nc.sync.dma_start(
                            it[:, :size // 16],
                            idx_d[:, (int(boff[b]) + start) // 16:
                                  (int(boff[b]) + start + size) // 16])
                        mt = msgp.tile([P, CH // P, P], bf16,
                                       name=f"mt{lname}", tag="msgs")
                        nc.gpsimd.dma_gather(
                            mt[:, :nb, :], tables[b][:],
                            it[:, :size // 16],
                            num_idxs=size, num_idxs_reg=size, elem_size=HID,
                            single_packet=False, queue_num=qn[0] % 4)
                        qn[0] += 1
                        # fold norm into messages (in place; no DVE-DVE RAW)
                        c0 = ci * (CH // P)
                        nt_b = nts[b][:, c0:c0 + nb].rearrange(
                            "p (b x) -> p b x", x=1).broadcast_to([P, nb, P])
                        nc.vector.tensor_tensor(mt[:, :nb, :], mt[:, :nb, :],
                                                nt_b, op=mybir.AluOpType.mult)
                    # pure one-hot: eq[p, i, j] = (j == lt[p, c0+i])
                    c0 = ci * (CH // P)
                    iota_b = iota[:].rearrange(
                        "p (b x) -> p b x", b=1).broadcast_to([P, nb, P])
                    lt_b = lts[b][:, c0:c0 + nb].rearrange(
                        "p (b x) -> p b x", x=1).broadcast_to([P, nb, P])
                    eq = eqp.tile([P, CH // P, P], bf16, name=f"eq{lname}",
                                  tag="eq")
                    nc.vector.tensor_tensor(eq[:, :nb, :], iota_b, lt_b,
                                            op=mybir.AluOpType.is_equal)
                    caches[b][ci] = (mt, eq)
                    return caches[b][ci]

                NG = 4                                     # windows per bank
                for g0 in range(0, W, NG):
                    gw = min(NG, W - g0)
                    pw = psp.tile([P, NG * P], f32, name=f"pw{lname}", tag="ps")
                    for j in range(gw):
                        wi = g0 + j
                        nbatch = int(Kmax[:, wi].sum())
                        assert nbatch > 0
                        done = 0
                        for b in range(NBK):
                            posb0 = int(dstoff[b, wi]) - int(boff[b])
                            for k in range(int(Kmax[b, wi])):
                                pos = posb0 + k * P
                                mt, eq = get_chunk(b, pos // CH)
                                i_in = (pos % CH) // P
                                lhs = (iota[:] if _KSUB in ("b", "c")
                                       else mt[:, i_in, :])
                                rhs = (eq[:, i_in, :] if _KSUB != "c"
                                       else iota[:])
                                nc.tensor.matmul(
                                    pw[:, j * P:(j + 1) * P], lhs, rhs,
                                    start=(done == 0),
                                    stop=(done == nbatch - 1))
                                done += 1
                    osl = xT_out[:, g0 * P:(g0 + gw) * P]
                    if bias_nz[bias_idx]:
                        nc.vector.tensor_scalar(
                            osl, pw[:, :gw * P],
                            biases[:, bias_idx:bias_idx + 1], None,
                            op0=mybir.AluOpType.add)
                    else:
                        nc.vector.tensor_copy(osl, pw[:, :gw * P])

            # ---- table build for layer 2 (+ per-bucket AllGather)
            def build_table2(xT, w_g, lname):
                hloc = [dramp.tile([BSL, HID], bf16, name=f"hloc{lname}_{b}")
                        for b in range(NBK)]
                table = [dramp.tile([BS, HID], bf16, addr_space="Shared",
                                    name=f"table{lname}_{b}")
                         for b in range(NBK)]
                if _KSUB == "j":
                    return table
                for b in range(NBK):
                    for (c0, nn) in _bucket_chunks(b):
                        ph = psp.tile([P, NCHUNK], f32, name=f"ph{lname}",
                                      tag="ps")
                        for s in range(nn // P):
                            nc.tensor.matmul(ph[:, s * P:(s + 1) * P],
                                             xT[:, c0 + s * P:c0 + (s + 1) * P],
                                             w_g, start=True, stop=True)
                        g = nn // P
                        hs = stgp.tile([P, NCHUNK // P, P], bf16,
                                       name=f"hs{lname}", tag="hs")
                        nc.vector.tensor_copy(
                            hs[:, :g, :].rearrange("p a b -> p (a b)"),
                            ph[:, :nn])
                        l0 = c0 - b * BSL
                        nc.sync.dma_start(
                            hloc[b][l0:l0 + nn, :].rearrange(
                                "(g p) f -> p g f", p=P),
                            hs[:, :g, :])
                    if _KSUB not in ("i", "j"):
                        nc.gpsimd.collective_compute(
                            "AllGather", mybir.AluOpType.bypass,
                            ins=[hloc[b].opt()], outs=[table[b].opt()],
                            replica_groups=[list(range(NCORES))],
                        )
                return table

            x3T = x1T
            if _KSTAGE >= 3:
                x2T = xtp.tile([P, SH], bf16, name="x2T", tag="xT")
                gcn_layer(table1, x2T, 2, "1")
                x3T = x2T
            if _KSTAGE >= 4:
                table2 = build_table2(x2T, w_g2, "2")
            if _KSTAGE >= 5:
                x3T = xtp.tile([P, SH], bf16, name="x3T", tag="xT")
                gcn_layer(table2, x3T, 3, "2")

            # ---- head: out = W_o2.T @ lrelu(W_o1.T @ x3T + b_o1) + b_o2
            for b in range(NBK):
                for (c0, nn) in _bucket_chunks(b):
                    py = psp.tile([P, NCHUNK], f32, name="py", tag="ps")
                    nc.tensor.matmul(py[:, :nn], w_o1, x3T[:, c0:c0 + nn],
                                     start=True, stop=True)
                    y = tmpp.tile([P, NCHUNK], bf16, name="y", tag="x0")
                    lrelu(y[:, :nn], py[:, :nn], 4, nn, "hd")
                    po = psp.tile([2, NCHUNK], f32, name="po", tag="ps")
                    nc.tensor.matmul(po[:, :nn], wo2[:], y[:, :nn],
                                     start=True, stop=True)
                    ostg = stgp.tile([2, NCHUNK], f32, name="ostg", tag="ostg")
                    if bias_nz[5]:
                        nc.vector.tensor_scalar(
                            ostg[:, :nn], po[:, :nn], biases[:2, 5:6], None,
                            op0=mybir.AluOpType.add)
                    else:
                        nc.vector.tensor_copy(ostg[:, :nn], po[:, :nn])
                    nc.sync.dma_start(out_d[:, c0:c0 + nn], ostg[:, :nn])

    nc.compile()
    return nc


# ---------------------------------------------------------------- entry

_CACHE = {}
_LAST_PP = None
_LAST_INMAPS = None


def _run(inputs, trace=False):
    global _LAST_PP, _LAST_INMAPS
    from concourse.bass_utils import run_bass_kernel_spmd

    pp = _preprocess(inputs["des"], inputs["tweet"], inputs["num_prop"],
                     inputs["cat_prop"], inputs["edge_index"])
    wblk, wsq, wo2, biases, bias_nz, iota = _weight_blobs(
        pp, inputs["W_des"], inputs["b_des"], inputs["W_tweet"], inputs["b_tweet"],
        inputs["W_num"], inputs["b_num"], inputs["W_cat"], inputs["b_cat"],
        inputs["W_in"], inputs["b_in"], inputs["W_g1"], inputs["b_g1"],
        inputs["W_g2"], inputs["b_g2"], inputs["W_o1"], inputs["b_o1"],
        inputs["W_o2"], inputs["b_o2"])

    key = (pp["N"], pp["E"], tuple(bias_nz), pp["L"], pp["B_tot"])
    if key not in _CACHE:
        _CACHE[key] = _build_nc(pp, bias_nz)
    nc = _CACHE[key]

    in_maps = []
    for c in range(NCORES):
        in_maps.append({
            "featT": pp["featT"][c], "idx": pp["idx"][c],
            "ldst": pp["ldst"][c], "nrm": pp["nrm"][c],
            "wblk": wblk, "wsq": wsq, "wo2": wo2, "biases": biases,
            "iota": iota,
        })
    _LAST_PP, _LAST_INMAPS = pp, in_maps
    res = run_bass_kernel_spmd(nc, in_maps, core_ids=list(range(NCORES)),
                               trace=trace)

    N = pp["N"]
    out_new = np.concatenate(
        [res.results[c]["out"].T for c in range(NCORES)], axis=0)  # [NP, 2]
    out = out_new[pp["new_id"], :]                                  # [N, 2]
    return np.ascontiguousarray(out.astype(np.float32)), res


def kernel(**inputs) -> np.ndarray:
    out, _ = _run(inputs, trace=False)
    return out
